# revision 1
# baseline (speedup 1.0000x reference)
"""Trainium2 Bass kernel for nn_GatedSpikingReservoirStep.

Reference computation (per batch row):
    prev = prev_state[:, :2048]
    input_part = inputs @ W_in.T                    # [B, R]
    reservoir_part = prev @ W_res.T                 # [B, R]
    gate = sigmoid(inputs @ W_gate.T)               # [B, 3R] -> i, f, o
    state = 0.9 * f * prev + 0.1 * tanh(i * (input_part + reservoir_part))
    state = o * state
    state = where(state > 0.5, state - 0.5, state)
    out = pad(state, [B, 2560])

Strategy: data-parallel over batch (8 cores x 512 rows). All matmuls are
computed transposed (out[r, b] = W_slice @ x_shard.T) so the contraction
dim (d or r') is the SBUF partition dim for both operands. The host
pre-packs every operand so each device DMA is per-partition contiguous.
Matmuls run in float32r (TF32-like multiply, fp32 accumulate, 4x the
fp32 rate). The gate/tanh/spike epilogue runs on ACT + DVE overlapped
with the next reservoir tile's matmuls.
"""

import numpy as np

B = 4096
D = 1024
R = 2048
MAX_DIM = 2560
N_CORES = 8
BS = B // N_CORES          # 512 batch rows per core
RT = R // 128              # 16 reservoir tiles of 128
KD = D // 128              # 8 contraction chunks over input dim
KR = R // 128              # 16 contraction chunks over reservoir dim

LEAK = 0.1
THRESH = 0.5

# 'f32r' (fast, ~1.5e-4 matmul rel err) or 'f32' (exact, 4x slower)
MM_MODE = 'f32r'

_cache = {}


def _build_nc():
    """Build and compile the per-core Bass module (same NEFF on all cores)."""
    import concourse.mybir as mybir
    import concourse.tile as tile
    from concourse import bacc

    F32 = mybir.dt.float32
    MMDT = mybir.dt.float32r if MM_MODE == 'f32r' else mybir.dt.float32
    AF = mybir.ActivationFunctionType
    OP = mybir.AluOpType

    nc = bacc.Bacc("TRN2", target_bir_lowering=False, debug=False)

    # Host-packed inputs; all are [128-partition, contiguous-free] blocks.
    x_d = nc.dram_tensor("x", [128, KD, BS], MMDT, kind="ExternalInput")
    p_d = nc.dram_tensor("p", [128, KR, BS], MMDT, kind="ExternalInput")
    win_d = nc.dram_tensor("win", [RT, 128, KD, 128], MMDT, kind="ExternalInput")
    wres_d = nc.dram_tensor("wres", [RT, 128, KR, 128], MMDT, kind="ExternalInput")
    wg_d = nc.dram_tensor("wg", [3, RT, 128, KD, 128], MMDT, kind="ExternalInput")
    out_d = nc.dram_tensor("out", [R, BS], F32, kind="ExternalOutput")

    with tile.TileContext(nc) as tc:
        with (
            tc.tile_pool(name="acts", bufs=1) as acts,
            tc.tile_pool(name="wpool", bufs=3) as wpool,
            tc.tile_pool(name="epi", bufs=2) as epi,
            tc.tile_pool(name="psum", bufs=2, space="PSUM") as psum,
        ):
            x_sb = acts.tile([128, KD, BS], MMDT)
            p_sb = acts.tile([128, KR, BS], MMDT)
            nc.sync.dma_start(x_sb[:], x_d.ap()[:])
            nc.sync.dma_start(p_sb[:], p_d.ap()[:])

            for t in range(RT):
                win_t = wpool.tile([128, KD, 128], MMDT, tag="win")
                wres_t = wpool.tile([128, KR, 128], MMDT, tag="wres")
                wg_t = wpool.tile([128, 3, KD, 128], MMDT, tag="wg")
                nc.sync.dma_start(win_t[:], win_d.ap()[t])
                nc.scalar.dma_start(wres_t[:], wres_d.ap()[t])
                for g in range(3):
                    nc.sync.dma_start(wg_t[:, g], wg_d.ap()[g, t])

                ps_i = psum.tile([128, BS], F32, tag="ps_i")
                ps_s = psum.tile([128, BS], F32, tag="ps_s")
                ps_f = psum.tile([128, BS], F32, tag="ps_f")
                ps_o = psum.tile([128, BS], F32, tag="ps_o")

                # i-gate logits first: unblocks the epilogue's longest chain
                for k in range(KD):
                    nc.tensor.matmul(ps_i[:], wg_t[:, 0, k], x_sb[:, k],
                                     start=(k == 0), stop=(k == KD - 1))
                # input_part + reservoir_part accumulate into one bank
                for k in range(KD):
                    nc.tensor.matmul(ps_s[:], win_t[:, k], x_sb[:, k],
                                     start=(k == 0), stop=False)
                for k in range(KR):
                    nc.tensor.matmul(ps_s[:], wres_t[:, k], p_sb[:, k],
                                     start=False, stop=(k == KR - 1))
                for k in range(KD):
                    nc.tensor.matmul(ps_f[:], wg_t[:, 1, k], x_sb[:, k],
                                     start=(k == 0), stop=(k == KD - 1))
                for k in range(KD):
                    nc.tensor.matmul(ps_o[:], wg_t[:, 2, k], x_sb[:, k],
                                     start=(k == 0), stop=(k == KD - 1))

                # epilogue: state = o*(0.9*f*prev + 0.1*tanh(i*(s))), spike
                prev_t = p_sb[:, t]
                if MM_MODE == 'f32r':
                    prev_t = prev_t.bitcast(F32)
                si = epi.tile([128, BS], F32, tag="si")
                nc.scalar.activation(si[:], ps_i[:], AF.Sigmoid)
                x1 = epi.tile([128, BS], F32, tag="x1")
                nc.vector.tensor_tensor(x1[:], si[:], ps_s[:], OP.mult)
                th = epi.tile([128, BS], F32, tag="th")
                nc.scalar.activation(th[:], x1[:], AF.Tanh)
                sf = epi.tile([128, BS], F32, tag="sf")
                nc.scalar.activation(sf[:], ps_f[:], AF.Sigmoid)
                fp9 = epi.tile([128, BS], F32, tag="fp9")
                nc.vector.scalar_tensor_tensor(fp9[:], sf[:], 1.0 - LEAK, prev_t,
                                               OP.mult, OP.mult)
                pre = epi.tile([128, BS], F32, tag="pre")
                nc.vector.scalar_tensor_tensor(pre[:], th[:], LEAK, fp9[:],
                                               OP.mult, OP.add)
                so = epi.tile([128, BS], F32, tag="so")
                nc.scalar.activation(so[:], ps_o[:], AF.Sigmoid)
                st = epi.tile([128, BS], F32, tag="st")
                nc.vector.tensor_tensor(st[:], pre[:], so[:], OP.mult)
                msk = epi.tile([128, BS], F32, tag="msk")
                nc.vector.tensor_scalar(msk[:], st[:], THRESH, THRESH,
                                        OP.is_gt, OP.mult)
                ot = epi.tile([128, BS], F32, tag="ot")
                nc.vector.tensor_tensor(ot[:], st[:], msk[:], OP.subtract)
                nc.gpsimd.dma_start(out_d.ap()[t * 128:(t + 1) * 128], ot[:])

    nc.compile()
    return nc


def _get_nc():
    if 'nc' not in _cache:
        _cache['nc'] = _build_nc()
    return _cache['nc']


def _pack_inputs(inputs, prev_state, W_in, W_res, W_gate):
    """Host-side packing: transpose so contraction dim lands on SBUF
    partitions, with per-partition-contiguous DMA blocks."""
    f = np.float32
    # x[c, p, k, b] = inputs[512c + b, 128k + p]
    xp = np.ascontiguousarray(
        inputs.reshape(N_CORES, BS, KD, 128).transpose(0, 3, 2, 1).astype(f, copy=False))
    # p[c, p, k, b] = prev_state[512c + b, 128k + p]
    pp = np.ascontiguousarray(
        prev_state[:, :R].reshape(N_CORES, BS, KR, 128).transpose(0, 3, 2, 1).astype(f, copy=False))
    # win[t, p, k, m] = W_in[128t + m, 128k + p]
    win = np.ascontiguousarray(
        W_in.reshape(RT, 128, KD, 128).transpose(0, 3, 2, 1).astype(f, copy=False))
    # wres[t, p, j, m] = W_res[128t + m, 128j + p]
    wres = np.ascontiguousarray(
        W_res.reshape(RT, 128, KR, 128).transpose(0, 3, 2, 1).astype(f, copy=False))
    # wg[g, t, p, k, m] = W_gate[2048g + 128t + m, 128k + p]
    wg = np.ascontiguousarray(
        W_gate.reshape(3, RT, 128, KD, 128).transpose(0, 1, 4, 3, 2).astype(f, copy=False))

    in_maps = []
    for c in range(N_CORES):
        in_maps.append({
            "x": xp[c], "p": pp[c],
            "win": win, "wres": wres, "wg": wg,
        })
    return in_maps


def _assemble(results):
    out = np.zeros((B, MAX_DIM), dtype=np.float32)
    for c in range(N_CORES):
        out[c * BS:(c + 1) * BS, :R] = results[c]["out"].T
    return out


def _run(in_maps, **spmd_kwargs):
    from concourse.bass_utils import run_bass_kernel_spmd
    nc = _get_nc()
    return run_bass_kernel_spmd(nc, in_maps, core_ids=list(range(N_CORES)),
                                **spmd_kwargs)


def kernel(inputs, prev_state, W_in, W_res, W_gate):
    in_maps = _pack_inputs(inputs, prev_state, W_in, W_res, W_gate)
    res = _run(in_maps)
    return _assemble(res.results)
